# revision 8
# baseline (speedup 1.0000x reference)
"""Trainium2 Bass kernel for nn_Attention_43413529428606 (linear attention
with l2-normed q/k, interleaved RoPE, mask, per-head power scaling).

Sparse/gathered sharding: the mask zeroes ~50% of rows on both the k/v
side (they contribute nothing to the k^T v state) and the q side (masked
rows emit zero output).  The host gathers only the unmasked rows of each
batch, splits them across 4 cores per batch (cores 0-3 batch 0, 4-7
batch 1), padding to a static CAP=1152 rows per core.  Each core
projects q/k/v for its gathered rows (all 16 heads), applies
l2norm+RoPE, accumulates the per-head k^T v state, AllReduces that
state (512 KB) within its batch group, then applies attention and the
output projection.  Padded rows carry x=0 so they vanish from the state
(an eps inside the Sqrt keeps the norm chain finite), and their output
columns are dropped at scatter time.  Masked rows are zero-filled on the
host.  Data path fp16 with fp32 PSUM accumulation.

Self-contained: hardcodes all shapes; no sibling imports.
"""

import sys

for _p in ("/opt/trn_rl_repo",):
    if _p not in sys.path:
        sys.path.append(_p)

from contextlib import ExitStack

import numpy as np

import concourse.bass as bass
import concourse.bacc as bacc
import concourse.tile as tile
from concourse import mybir
from concourse.bass_utils import run_bass_kernel_spmd

F32 = mybir.dt.float32
F16 = mybir.dt.float16

DIM = 1024
H = 16
HD = 64
B = 2
C = 8192
ROPE_THETA = 10000.0

N_CORES = 8
CAP = 1152  # static per-core row capacity (binomial max ~1060)
NC_T = CAP // 128  # 9 c-tiles of 128 (phase A)
ST = 384  # phase B supertile width
NQ_T = CAP // ST  # 3 c-supertiles
ND = DIM // 128  # 8 d-chunks
NJ = DIM // 128  # 8 j-tiles
NPAIR = H // 2  # 8 head pairs

Copy = mybir.ActivationFunctionType.Copy
Square = mybir.ActivationFunctionType.Square
Sqrt = mybir.ActivationFunctionType.Sqrt
MUL = mybir.AluOpType.mult
ADD = mybir.AluOpType.add

EPS = 1e-9


def build_nc(sim_mode=False, reps=1):
    nc = bacc.Bacc(
        "TRN2",
        target_bir_lowering=False,
        debug=False,
        num_devices=1 if sim_mode else N_CORES,
    )

    # ---- DRAM parameters (per-core shapes, fp16 data path) ----
    xT = nc.dram_tensor("xT", [DIM, CAP], F16, kind="ExternalInput").ap()
    WkT = nc.dram_tensor("WkT", [DIM, DIM], F16, kind="ExternalInput").ap()
    WvT = nc.dram_tensor("WvT", [DIM, DIM], F16, kind="ExternalInput").ap()
    WqT = nc.dram_tensor("WqT", [DIM, DIM], F16, kind="ExternalInput").ap()
    WoT = nc.dram_tensor("WoT", [DIM, DIM], F16, kind="ExternalInput").ap()
    cosC = nc.dram_tensor("cosC", [CAP, HD], F16, kind="ExternalInput").ap()
    sinC = nc.dram_tensor("sinC", [CAP, HD], F16, kind="ExternalInput").ap()
    cosF = nc.dram_tensor("cosF", [128, CAP], F16, kind="ExternalInput").ap()
    sinF = nc.dram_tensor("sinF", [128, CAP], F16, kind="ExternalInput").ap()
    ind16T = nc.dram_tensor("ind16T", [DIM, 16], F16, kind="ExternalInput").ap()
    ind16 = nc.dram_tensor("ind16", [16, DIM], F16, kind="ExternalInput").ap()
    Pmat = nc.dram_tensor("Pmat", [128, 128], F16, kind="ExternalInput").ap()

    kv_in_d = nc.dram_tensor("kv_in_d", [128, NPAIR * 128], F32)
    kv_out_d = nc.dram_tensor("kv_out_d", [128, NPAIR * 128], F32)

    out_d = nc.dram_tensor("out", [DIM, CAP], F32, kind="ExternalOutput").ap()

    with tile.TileContext(nc) as tc:
        with ExitStack() as ctx:
            consts = ctx.enter_context(tc.tile_pool(name="consts", bufs=1))
            kvblk_pool = ctx.enter_context(tc.tile_pool(name="kvblk", bufs=1))

            # rope tables + index matrices on the vector DMA queue
            cosC_t = consts.tile([128, NC_T * HD], F16, tag="cosC")
            sinC_t = consts.tile([128, NC_T * HD], F16, tag="sinC")
            nc.gpsimd.dma_start(
                out=cosC_t[:].rearrange("p (t f) -> p t f", t=NC_T),
                in_=cosC[:].rearrange("(t p) f -> p t f", p=128),
            )
            nc.gpsimd.dma_start(
                out=sinC_t[:].rearrange("p (t f) -> p t f", t=NC_T),
                in_=sinC[:].rearrange("(t p) f -> p t f", p=128),
            )
            cosF_t = consts.tile([128, CAP], F16, tag="cosF")
            sinF_t = consts.tile([128, CAP], F16, tag="sinF")
            nc.gpsimd.dma_start(out=cosF_t[:], in_=cosF[:])
            nc.gpsimd.dma_start(out=sinF_t[:], in_=sinF[:])
            ind16T_t = consts.tile([128, NJ * 16], F16, tag="ind16T")
            ind16_t = consts.tile([16, DIM], F16, tag="ind16")
            P_t = consts.tile([128, 128], F16, tag="Pmat")
            nc.gpsimd.dma_start(
                out=ind16T_t[:].rearrange("p (t f) -> p t f", t=NJ),
                in_=ind16T[:].rearrange("(t p) f -> p t f", p=128),
            )
            nc.gpsimd.dma_start(out=ind16_t[:], in_=ind16[:])
            nc.gpsimd.dma_start(out=P_t[:], in_=Pmat[:])
            eps_t = consts.tile([128, 1], F32, tag="eps")
            nc.vector.memset(eps_t[:], EPS)

            for _rep in range(reps):
              with ExitStack() as ctxX:
                xpool = ctxX.enter_context(tc.tile_pool(name="xpool", bufs=1))
                wBC = ctxX.enter_context(tc.tile_pool(name="wBC", bufs=1))

                # x loaded per c-tile column slice so tile-0 matmuls can
                # start after ~256KB instead of the whole 2.25MB
                xT_all = xpool.tile([128, ND * CAP], F16, tag="xT")
                xT_v = xT_all[:].rearrange("p (t c) -> p t c", t=ND)
                xT_src = xT.rearrange("(t p) c -> p t c", p=128)
                for ct in range(NC_T):
                    cs = slice(ct * 128, (ct + 1) * 128)
                    nc.sync.dma_start(out=xT_v[:, :, cs], in_=xT_src[:, :, cs])

                def xsl(dc, csl):
                    lo = dc * CAP
                    return xT_all[:, lo + csl.start : lo + csl.stop]

                wq_all = wBC.tile([128, ND * DIM], F16, tag="wq")
                wo_all = wBC.tile([128, ND * DIM], F16, tag="wo")

                # ========= Phase A: k/v proj + process + kv Grams ==========
                with ExitStack() as ctxA:
                    wA = ctxA.enter_context(tc.tile_pool(name="wA", bufs=1))
                    psA = ctxA.enter_context(
                        tc.tile_pool(name="psA", bufs=3, space="PSUM")
                    )
                    pskv = ctxA.enter_context(
                        tc.tile_pool(name="pskv", bufs=1, space="PSUM")
                    )
                    sbA = ctxA.enter_context(tc.tile_pool(name="sbA", bufs=2))
                    sb1 = ctxA.enter_context(tc.tile_pool(name="sb1", bufs=2))
                    smA = ctxA.enter_context(tc.tile_pool(name="smA", bufs=2))

                    wk_all = wA.tile([128, ND * DIM], F16, tag="wk")
                    wv_all = wA.tile([128, ND * DIM], F16, tag="wv")
                    # k first (phase A k-matmuls come first), then v, then
                    # the phase-B weights on the same queue
                    for wt, wsrc in (
                        (wk_all, WkT),
                        (wv_all, WvT),
                        (wq_all, WqT),
                        (wo_all, WoT),
                    ):
                        for xc in range(2):
                            nc.scalar.dma_start(
                                out=wt[
                                    :, xc * 4 * DIM : (xc + 1) * 4 * DIM
                                ].rearrange("p (t f) -> p t f", t=4),
                                in_=wsrc[xc * 512 : (xc + 1) * 512, :].rearrange(
                                    "(t p) f -> p t f", p=128
                                ),
                            )

                    kv_ps = pskv.tile([128, NPAIR * 128], F32, tag="kvps")
                    kv_pending = []

                    # On HW start=True zeroes the whole PSUM bank, so only
                    # the first pair written to each bank may carry it.
                    def _emit_kv(item):
                        ct_, khat_, v_ = item
                        for p in range(NPAIR):
                            ps_ = slice(p * 128, (p + 1) * 128)
                            nc.tensor.matmul(
                                kv_ps[:, ps_],
                                khat_[:, ps_],
                                v_[:, ps_],
                                start=(
                                    True
                                    if sim_mode
                                    else (ct_ == 0 and p % 4 == 0)
                                ),
                                stop=(
                                    True if sim_mode else (ct_ == NC_T - 1)
                                ),
                            )

                    for ct in range(NC_T):
                        cs = slice(ct * 128, (ct + 1) * 128)
                        k_ps = psA.tile([128, DIM], F32, tag="proj_ps")
                        v_ps = psA.tile([128, DIM], F32, tag="proj_ps")
                        for half in range(2):
                            js = slice(half * 512, (half + 1) * 512)
                            for dc in range(ND):
                                nc.tensor.matmul(
                                    k_ps[:, js],
                                    xsl(dc, cs),
                                    wk_all[
                                        :, dc * DIM + js.start : dc * DIM + js.stop
                                    ],
                                    start=(dc == 0),
                                    stop=(dc == ND - 1),
                                )
                            for dc in range(ND):
                                nc.tensor.matmul(
                                    v_ps[:, js],
                                    xsl(dc, cs),
                                    wv_all[
                                        :, dc * DIM + js.start : dc * DIM + js.stop
                                    ],
                                    start=(dc == 0),
                                    stop=(dc == ND - 1),
                                )

                        # evictions on ACT (one func-set: Copy/Square/Sqrt);
                        # k first so the norm/rope chain starts asap
                        k_sb = sbA.tile([128, DIM], F16, tag="k_sb")
                        nc.scalar.activation(k_sb[:], k_ps[:], Copy)
                        sq = sbA.tile([128, DIM], F16, tag="sq")
                        nc.scalar.activation(sq[:], k_ps[:], Square)
                        v_sb = sbA.tile([128, DIM], F16, tag="v_sb")
                        nc.scalar.activation(v_sb[:], v_ps[:], Copy)

                        # per-head sumsq -> rsqrt (eps keeps padded rows finite)
                        red = smA.tile([128, H], F32, tag="red")
                        nc.vector.tensor_reduce(
                            red[:],
                            sq[:].rearrange("p (h f) -> p h f", h=H),
                            mybir.AxisListType.X,
                            ADD,
                        )
                        sqr = smA.tile([128, H], F32, tag="sqr")
                        nc.scalar.activation(sqr[:], red[:], Sqrt, bias=eps_t[:])
                        rsf = smA.tile([128, H], F32, tag="rsf")
                        nc.vector.reciprocal(rsf[:], sqr[:])

                        # rope: m1 on DVE, strided swap-mul on GPSIMD
                        cosb = (
                            cosC_t[:, ct * HD : (ct + 1) * HD]
                            .unsqueeze(1)
                            .broadcast_to([128, H, HD])
                        )
                        sinb4 = (
                            sinC_t[:, ct * HD : (ct + 1) * HD]
                            .rearrange("p (g two) -> p g two", two=2)
                            .unsqueeze(1)
                            .broadcast_to([128, H, HD // 2, 2])
                        )
                        m1 = sb1.tile([128, DIM], F16, tag="m1")
                        nc.vector.tensor_tensor(
                            m1[:].rearrange("p (h f) -> p h f", h=H),
                            k_sb[:].rearrange("p (h f) -> p h f", h=H),
                            cosb,
                            MUL,
                        )
                        k_sw = k_sb[:].rearrange(
                            "p (h g two) -> p h g two", h=H, two=2
                        )[:, :, :, ::-1]
                        m2 = sb1.tile([128, DIM], F16, tag="m2")
                        nc.gpsimd.tensor_tensor(
                            m2[:].rearrange("p (h g two) -> p h g two", h=H, two=2),
                            k_sw,
                            sinb4,
                            MUL,
                        )
                        s = sb1.tile([128, DIM], F16, tag="s")
                        nc.vector.tensor_tensor(s[:], m1[:], m2[:], ADD)
                        khat = sbA.tile([128, DIM], F16, tag="khat")
                        rsb = rsf[:].unsqueeze(2).broadcast_to([128, H, HD])
                        nc.vector.tensor_tensor(
                            khat[:].rearrange("p (h f) -> p h f", h=H),
                            s[:].rearrange("p (h f) -> p h f", h=H),
                            rsb,
                            MUL,
                        )

                        # kv Grams issued one iteration late (software
                        # pipelining) so PE never waits on the khat chain
                        kv_pending.append((ct, khat, v_sb))
                        if len(kv_pending) > 1:
                            _emit_kv(kv_pending.pop(0))

                    while kv_pending:
                        _emit_kv(kv_pending.pop(0))

                    # evict kv partials and run the collective
                    kv_sb = sbA.tile([128, NPAIR * 128], F32, tag="kv_sb")
                    nc.vector.tensor_copy(kv_sb[:], kv_ps[:])
                    nc.sync.dma_start(out=kv_in_d.ap(), in_=kv_sb[:])
                    if sim_mode:
                        # stand-in for the AllReduce so TimelineSim can run
                        nc.sync.dma_start(out=kv_out_d.ap(), in_=kv_in_d.ap())
                    else:
                        nc.gpsimd.collective_compute(
                            "AllReduce",
                            ADD,
                            replica_groups=[[0, 1, 2, 3], [4, 5, 6, 7]],
                            ins=[kv_in_d.ap().opt()],
                            outs=[kv_out_d.ap().opt()],
                        )

                # kvblk: load reduced Grams, cast to fp16 block-diag
                kvblk = kvblk_pool.tile([128, NPAIR * 128], F16, tag="kvblk")
                kvf = kvblk_pool.tile([128, NPAIR * 128], F32, tag="kvf")
                nc.scalar.dma_start(out=kvf[:], in_=kv_out_d.ap())
                nc.vector.memset(kvblk[:], 0.0)
                # top-left diag blocks of each pair, then bottom-right
                nc.vector.tensor_copy(
                    kvblk[0:64, :].rearrange("p (t f) -> p t f", t=NPAIR)[
                        :, :, 0:64
                    ],
                    kvf[0:64, :].rearrange("p (t f) -> p t f", t=NPAIR)[
                        :, :, 0:64
                    ],
                )
                nc.vector.tensor_copy(
                    kvblk[64:128, :].rearrange("p (t f) -> p t f", t=NPAIR)[
                        :, :, 64:128
                    ],
                    kvf[64:128, :].rearrange("p (t f) -> p t f", t=NPAIR)[
                        :, :, 64:128
                    ],
                )

                # ==== Fused phase B+C: q proj/norm/rope + attn + out proj ===
                with ExitStack() as ctxB:
                    psB = ctxB.enter_context(
                        tc.tile_pool(name="psB", bufs=2, space="PSUM")
                    )
                    psR = ctxB.enter_context(
                        tc.tile_pool(name="psR", bufs=3, space="PSUM")
                    )
                    psN = ctxB.enter_context(
                        tc.tile_pool(name="psN", bufs=1, space="PSUM")
                    )
                    psO = ctxB.enter_context(
                        tc.tile_pool(name="psO", bufs=2, space="PSUM")
                    )
                    sbB = ctxB.enter_context(tc.tile_pool(name="sbB", bufs=3))
                    sbS = ctxB.enter_context(
                        tc.tile_pool(name="sbS", bufs=2 * NJ)
                    )
                    sbQ = ctxB.enter_context(tc.tile_pool(name="sbQ", bufs=2))
                    sbAt = ctxB.enter_context(
                        tc.tile_pool(name="sbAt", bufs=NJ + 2)
                    )

                    def _emit_attn_out(item):
                        ct_, qh_ = item
                        cs_ = slice(ct_ * ST, (ct_ + 1) * ST)
                        attn_sb = []
                        for hp in range(NPAIR):
                            a_ps = psO.tile([128, ST], F32, tag="ao_ps")
                            nc.tensor.matmul(
                                a_ps[:],
                                kvblk[:, hp * 128 : (hp + 1) * 128],
                                qh_[:, hp * ST : (hp + 1) * ST],
                                start=True,
                                stop=True,
                            )
                            a_sb = sbAt.tile([128, ST], F16, tag="a_sb")
                            if hp % 2 == 0:
                                nc.scalar.activation(a_sb[:], a_ps[:], Copy)
                            else:
                                nc.vector.tensor_copy(a_sb[:], a_ps[:])
                            attn_sb.append(a_sb)

                        for et in range(NJ):
                            elo = et * 128
                            o_ps = psO.tile([128, ST], F32, tag="ao_ps")
                            for jt in range(NJ):
                                nc.tensor.matmul(
                                    o_ps[:],
                                    wo_all[
                                        :, jt * DIM + elo : jt * DIM + elo + 128
                                    ],
                                    attn_sb[jt][:],
                                    start=(jt == 0),
                                    stop=(jt == NJ - 1),
                                )
                            o_sb = sbB.tile([128, ST], F32, tag="o_sb")
                            nc.scalar.activation(o_sb[:], o_ps[:], Copy)
                            nc.sync.dma_start(
                                out=out_d.rearrange("(t p) c -> p t c", p=128)[
                                    :, et, cs_
                                ],
                                in_=o_sb[:],
                            )

                    at_pending = []
                    for ct in range(NQ_T):
                        cs = slice(ct * ST, (ct + 1) * ST)
                        norms_ps = psN.tile([16, ST], F32, tag="norms")
                        qh_all = sbQ.tile([128, NJ * ST], F16, tag="qhall")
                        q_sbs = []
                        # pass 1: projections + squares + norm accumulation
                        for jt in range(NJ):
                            jlo = jt * 128
                            q_ps = psB.tile([128, ST], F32, tag="q_ps")
                            for dc in range(ND):
                                nc.tensor.matmul(
                                    q_ps[:],
                                    wq_all[
                                        :, dc * DIM + jlo : dc * DIM + jlo + 128
                                    ],
                                    xsl(dc, cs),
                                    start=(dc == 0),
                                    stop=(dc == ND - 1),
                                )
                            q_sb = sbS.tile([128, ST], F16, tag="q_sb")
                            nc.scalar.activation(q_sb[:], q_ps[:], Copy)
                            sq = sbB.tile([128, ST], F16, tag="sqB")
                            nc.vector.tensor_mul(sq[:], q_sb[:], q_sb[:])
                            nc.tensor.matmul(
                                norms_ps[:],
                                ind16T_t[:, jt * 16 : (jt + 1) * 16],
                                sq[:],
                                start=(jt == 0),
                                stop=(jt == NJ - 1),
                            )
                            q_sbs.append(q_sb)

                        sq16 = sbB.tile([16, ST], F32, tag="sq16")
                        nc.scalar.activation(sq16[:], norms_ps[:], Sqrt, bias=eps_t[:16])
                        rsf16 = sbB.tile([16, ST], F32, tag="rsf16")
                        nc.vector.reciprocal(rsf16[:], sq16[:])
                        rs16 = sbB.tile([16, ST], F16, tag="rs16")
                        nc.vector.tensor_copy(rs16[:], rsf16[:])

                        # pass 2: rotation + rope + scale into qh_all
                        for jt in range(NJ):
                            q_sb = q_sbs[jt]
                            rot_ps = psR.tile([128, ST], F32, tag="rotrep")
                            nc.tensor.matmul(
                                rot_ps[:], P_t[:], q_sb[:], start=True, stop=True
                            )
                            rep_ps = psR.tile([128, ST], F32, tag="rotrep")
                            nc.tensor.matmul(
                                rep_ps[:],
                                ind16_t[:, jt * 128 : (jt + 1) * 128],
                                rs16[:],
                                start=True,
                                stop=True,
                            )
                            # evict both to SBUF f16 on ACT, then dense GPSIMD
                            # muls keep DVE at 2x mode
                            rot_sb = sbB.tile([128, ST], F16, tag="rot_sb")
                            nc.scalar.activation(rot_sb[:], rot_ps[:], Copy)
                            rep_sb = sbB.tile([128, ST], F16, tag="rep_sb")
                            nc.scalar.activation(rep_sb[:], rep_ps[:], Copy)
                            t1 = sbB.tile([128, ST], F16, tag="t1")
                            nc.vector.tensor_tensor(
                                t1[:], q_sb[:], cosF_t[:, cs], MUL
                            )
                            t2 = sbB.tile([128, ST], F16, tag="t2")
                            nc.gpsimd.tensor_tensor(
                                t2[:], rot_sb[:], sinF_t[:, cs], MUL
                            )
                            s = sbB.tile([128, ST], F16, tag="sB")
                            nc.vector.tensor_tensor(s[:], t1[:], t2[:], ADD)
                            nc.vector.tensor_tensor(
                                qh_all[:, jt * ST : (jt + 1) * ST],
                                s[:],
                                rep_sb[:],
                                MUL,
                            )

                        at_pending.append((ct, qh_all))
                        if len(at_pending) > 1:
                            _emit_attn_out(at_pending.pop(0))

                    while at_pending:
                        _emit_attn_out(at_pending.pop(0))

    nc.compile()
    return nc


_NC_CACHE = None


def _get_nc():
    global _NC_CACHE
    if _NC_CACHE is None:
        _NC_CACHE = build_nc()
    return _NC_CACHE


def make_in_maps(x, mask, Wq, Wk, Wv, Wo, norm_const):
    x = np.asarray(x, np.float32)
    mask = np.asarray(mask)
    Wq = np.asarray(Wq, np.float32)
    Wk = np.asarray(Wk, np.float32)
    Wv = np.asarray(Wv, np.float32)
    Wo = np.asarray(Wo, np.float32)
    norm_const = np.asarray(norm_const, np.float32).reshape(H)

    sig = 1.0 / (1.0 + np.exp(-norm_const.astype(np.float64)))
    svec = np.float64(C) ** (-sig)  # [H]
    s_cols = np.repeat(svec, HD)  # [DIM]

    f16 = np.float16
    WkT = np.ascontiguousarray(Wk.T).astype(f16)
    WvT = np.ascontiguousarray((Wv * s_cols[:, None].astype(np.float32)).T).astype(
        f16
    )
    WqT = np.ascontiguousarray(Wq.T).astype(f16)
    WoT = np.ascontiguousarray(Wo.T).astype(f16)

    inv_freq = 1.0 / (
        ROPE_THETA ** (np.arange(0, HD, 2, dtype=np.float64) / HD)
    )  # [32]
    freq_of_j = np.repeat(inv_freq, 2)  # [64] interleaved
    # sign fold for the swap formulation: even j -> -sin, odd j -> +sin
    sign_fold = np.where(np.arange(HD) % 2 == 0, -1.0, 1.0)

    ind16T = np.zeros((DIM, 16), f16)
    for jt in range(NJ):
        for kk in range(128):
            ind16T[jt * 128 + kk, 2 * jt + (kk >= 64)] = 1.0

    ind16 = np.zeros((16, DIM), f16)
    for jt in range(NJ):
        for m in range(128):
            ind16[2 * jt + (m >= 64), jt * 128 + m] = 1.0

    Pmat = np.zeros((128, 128), f16)
    for i in range(64):
        Pmat[2 * i + 1, 2 * i] = -1.0  # out[2i] = -q[2i+1]
        Pmat[2 * i, 2 * i + 1] = 1.0  # out[2i+1] = q[2i]

    in_maps = []
    metas = []
    for core in range(N_CORES):
        b = core // (N_CORES // B)
        cc = core % (N_CORES // B)
        idx = np.nonzero(mask[b])[0]
        n = len(idx)
        base, rem = divmod(n, N_CORES // B)
        cnt = base + (1 if cc < rem else 0)
        start = cc * base + min(cc, rem)
        rows = idx[start : start + cnt]
        assert cnt <= CAP, f"core {core}: {cnt} unmasked rows > CAP={CAP}"

        xg = np.zeros((CAP, DIM), np.float32)
        xg[:cnt] = x[b, rows, :]
        xTc = np.ascontiguousarray(xg.T).astype(f16)

        pos = np.zeros(CAP, np.float64)
        pos[:cnt] = rows

        angC = pos[:, None] * freq_of_j[None, :]  # [CAP, 64]
        cosCc = np.cos(angC).astype(f16)
        sinCc = (np.sin(angC) * sign_fold[None, :]).astype(f16)

        angF = freq_of_j[:, None] * pos[None, :]  # [64, CAP]
        angF2 = np.concatenate([angF, angF], axis=0)  # [128, CAP]
        cosFc = np.cos(angF2).astype(f16)
        sinFc = np.sin(angF2).astype(f16)

        in_maps.append(
            {
                "xT": xTc,
                "WkT": WkT,
                "WvT": WvT,
                "WqT": WqT,
                "WoT": WoT,
                "cosC": cosCc,
                "sinC": sinCc,
                "cosF": cosFc,
                "sinF": sinFc,
                "ind16T": ind16T,
                "ind16": ind16,
                "Pmat": Pmat,
            }
        )
        metas.append((b, rows))
    return in_maps, metas


def assemble_output(results, metas):
    out = np.zeros((B, C, DIM), np.float32)
    for core, (b, rows) in enumerate(metas):
        out[b, rows, :] = results[core]["out"].T[: len(rows)]
    return out


def kernel(x, mask, Wq, Wk, Wv, Wo, norm_const):
    nc = _get_nc()
    in_maps, metas = make_in_maps(x, mask, Wq, Wk, Wv, Wo, norm_const)
    res = run_bass_kernel_spmd(nc, in_maps, list(range(N_CORES)))
    return assemble_output(res.results, metas)
